# revision 41
# baseline (speedup 1.0000x reference)
"""Trainium2 Bass kernel for a 2-layer Mamba stack (BasicLayer).

Per layer: LayerNorm -> in_proj (1024->4096) -> causal depthwise conv(k=4)
+ SiLU -> x_proj (2048->96) -> dt_proj + softplus -> selective scan over
L=2048 -> gate with SiLU(z) -> out_proj (2048->1024).

Sharding: tensor-parallel over d_inner for everything up to the gate
(2048 / 8 cores = 256 channels per core; the selective scan is
independent per channel), then token-parallel for out_proj: the gated
activations are redistributed with one AllToAll per batch so each core
computes the full out_proj for its own 256-token slab (no output
AllReduce; the host assembles the slabs).  Between layers the
LayerNorm'd slabs are AllGathered in bf16.

Engine budget choices:
 - all matmuls in bf16 (fp32 matmuls are 4 cycles/row vs 1 for bf16)
 - the depthwise conv runs on the tensor engine as 4 PSUM-accumulated
   matmuls against per-tap diagonal matrices
 - one activation table (natural_log_exp) for the whole kernel: silu is
   computed as x*sigmoid(x) with sigmoid = exp(-softplus(-x)) on the
   scalar engine, softplus = Ln(1+Exp(x)), and LayerNorm's rsqrt as
   exp(-0.5*ln(var+eps)) -- zero activation-table reloads
 - the scan recurrence h_t = exp(dt*A)*h_{t-1} + (dt*u*B)_t runs on the
   DVE tensor_tensor_scan instruction in bf16 (fp32 internal state)
 - B_t / C_t rows are replicated across partitions with bf16 broadcast
   DMAs, shared across the channel tiles of a batch
"""

import numpy as np

try:
    import concourse.bass as bass
except ImportError:  # pragma: no cover - fallback for odd sys.path setups
    import sys

    sys.path.insert(0, "/opt/trn_rl_repo")
    import concourse.bass as bass

import concourse.bacc as bacc
import concourse.mybir as mybir
import concourse.tile as tile
from concourse.bass_utils import run_bass_kernel_spmd

F32 = mybir.dt.float32
BF16 = mybir.dt.bfloat16
AF = mybir.ActivationFunctionType
ALU = mybir.AluOpType

# Problem shapes (hardcoded per the contract)
B, L = 2, 2048
DM, DI, DS, DTR, DCONV, DEPTH = 1024, 2048, 16, 64, 4, 2
EPS = 1e-5
NCORES = 8
DL = DI // NCORES          # 256 channels per core
NDT = DL // 128            # 2 channel tiles per core
T = B * L                  # 4096 tokens
NCH = T // 512             # 8 chunks of 512 tokens
SLAB = L // NCORES         # 256 tokens per core per batch


_TABLES_PINNED = False


def _pin_act_tables():
    global _TABLES_PINNED
    if _TABLES_PINNED:
        return
    _TABLES_PINNED = True
    orig = bacc.get_activation_tables

    def only_lnexp(arch):
        t = orig(arch)
        return {k: (v if k == "natural_log_exp_and_others" else set())
                for k, v in t.items()}

    bacc.get_activation_tables = only_lnexp


def build_nc(apply_norm_w: bool, apply_norm_b: bool, fake_cc: bool = False):
    _pin_act_tables()
    nc = bacc.Bacc(
        "TRN2",
        target_bir_lowering=False,
        debug=False,
        enable_asserts=False,
        num_devices=NCORES,
    )

    # ---- I/O declarations (per-core data supplied via in_maps) ----
    x_dram = nc.dram_tensor("x_tm", [T, DM], F32, kind="ExternalInput")
    w_inT = nc.dram_tensor("w_inT", [DEPTH, DM, 4 * 128], BF16, kind="ExternalInput")
    w_outF = nc.dram_tensor("w_outF", [DEPTH, DI, DM], BF16, kind="ExternalInput")
    w_xpT = nc.dram_tensor("w_xpT", [DEPTH, DL, 96], BF16, kind="ExternalInput")
    w_dtT = nc.dram_tensor("w_dtT", [DEPTH, DTR, DL], BF16, kind="ExternalInput")
    conv_dg = nc.dram_tensor("conv_dg", [DEPTH, NDT, DCONV, 128, 128], BF16,
                             kind="ExternalInput")
    conv_b = nc.dram_tensor("conv_b_c", [DEPTH, DL, 1], F32, kind="ExternalInput")
    nconv_b = nc.dram_tensor("nconv_b_c", [DEPTH, DL, 1], F32, kind="ExternalInput")
    dt_b = nc.dram_tensor("dt_b_c", [DEPTH, DL, 1], F32, kind="ExternalInput")
    a_neg = nc.dram_tensor("a_neg_c", [DEPTH, DL, DS], F32, kind="ExternalInput")
    d_p = nc.dram_tensor("d_c", [DEPTH, DL, 1], F32, kind="ExternalInput")
    ident_bf = nc.dram_tensor("ident_bf", [128, 128], BF16, kind="ExternalInput")
    if apply_norm_w:
        nwb = nc.dram_tensor("norm_w_bc", [DEPTH, 128, DM], F32, kind="ExternalInput")
    if apply_norm_b:
        nbb = nc.dram_tensor("norm_b_bc", [DEPTH, 128, DM], F32, kind="ExternalInput")
    out_dram = nc.dram_tensor("out_tm", [B * SLAB, DM], F32, kind="ExternalOutput")

    groups = [list(range(NCORES))]

    def all_reduce(src_ap, dst_ap):
        if fake_cc:
            nc.sync.dma_start(dst_ap, src_ap)
        else:
            nc.gpsimd.collective_compute(
                "AllReduce", ALU.add, replica_groups=groups,
                ins=[src_ap], outs=[dst_ap],
            )

    def all_to_all(src_ap, dst_ap):
        if fake_cc:
            nc.sync.dma_start(dst_ap, src_ap)
        else:
            nc.gpsimd.collective_compute(
                "AllToAll", ALU.bypass, replica_groups=groups,
                ins=[src_ap], outs=[dst_ap],
            )

    def all_gather(src_ap, dst_ap):
        if fake_cc:
            nc.sync.dma_start(dst_ap, src_ap)
            return
        nc.gpsimd.collective_compute(
            "AllGather", ALU.bypass, replica_groups=groups,
            ins=[src_ap], outs=[dst_ap],
        )

    with tile.TileContext(nc, num_cores=NCORES) as tc:
        with (
            tc.tile_pool(name="wp", bufs=1) as wp,
            tc.tile_pool(name="lnp", bufs=1) as lnp,
            tc.tile_pool(name="sp", bufs=1) as sp,
            tc.tile_pool(name="dp", bufs=1) as dp,
            tc.tile_pool(name="psA", bufs=1, space="PSUM") as psA,
            tc.tile_pool(name="psE", bufs=1, space="PSUM") as psE,
            tc.tile_pool(name="dram", bufs=2, space="DRAM") as dram,
        ):
            idbf = wp.tile([128, 128], BF16, tag="idbf")
            nc.sync.dma_start(idbf[:], ident_bf[:, :])
            one_sb = wp.tile([128, 1], F32, tag="one")
            nc.vector.memset(one_sb[:], 1.0)
            eps_sb = wp.tile([128, 1], F32, tag="eps")
            nc.vector.memset(eps_sb[:], EPS)

            # ---- AllGather landing for layer-2 input (split per batch) ----
            hn_ag_in = [dram.tile([SLAB, DM], BF16, tag=f"agin{b}", name=f"agin{b}")
                        for b in range(B)]
            hn_ag = [dram.tile([NCORES * SLAB, DM], BF16, tag=f"agout{b}",
                               addr_space="Shared", name=f"agout{b}")
                     for b in range(B)]

            for l in range(DEPTH):
                # ================= per-layer weights =================
                winT = []
                for kt in range(8):
                    w = wp.tile([128, 512], BF16, tag=f"winT{kt}")
                    nc.sync.dma_start(w[:], w_inT[l, kt * 128:(kt + 1) * 128, :])
                    winT.append(w)
                wout = []
                for k in range(16):
                    w = wp.tile([128, DM], BF16, tag=f"wout{k}")
                    nc.sync.dma_start(w[:], w_outF[l, k * 128:(k + 1) * 128, :])
                    wout.append(w)
                wxpT = []
                for j in range(NDT):
                    w = wp.tile([128, 96], BF16, tag=f"wxpT{j}")
                    nc.sync.dma_start(w[:], w_xpT[l, j * 128:(j + 1) * 128, :])
                    wxpT.append(w)
                wdtT = wp.tile([DTR, DL], BF16, tag="wdtT")
                nc.sync.dma_start(wdtT[:], w_dtT[l, :, :])
                cdiag, convb, ncb, dtb, Dp, Asb = [], [], [], [], [], []
                for j in range(NDT):
                    dgs = []
                    for k in range(DCONV):
                        dg = wp.tile([128, 128], BF16, tag=f"cdg{j}_{k}")
                        nc.sync.dma_start(dg[:], conv_dg[l, j, k, :, :])
                        dgs.append(dg)
                    cdiag.append(dgs)
                    cb = wp.tile([128, 1], F32, tag=f"convb{j}")
                    nc.sync.dma_start(cb[:], conv_b[l, j * 128:(j + 1) * 128, :])
                    convb.append(cb)
                    nb = wp.tile([128, 1], F32, tag=f"nconvb{j}")
                    nc.sync.dma_start(nb[:], nconv_b[l, j * 128:(j + 1) * 128, :])
                    ncb.append(nb)
                    db = wp.tile([128, 1], F32, tag=f"dtb{j}")
                    nc.sync.dma_start(db[:], dt_b[l, j * 128:(j + 1) * 128, :])
                    dtb.append(db)
                    dd = wp.tile([128, 1], F32, tag=f"dd{j}")
                    nc.sync.dma_start(dd[:], d_p[l, j * 128:(j + 1) * 128, :])
                    Dp.append(dd)
                    at = wp.tile([128, DS], F32, tag=f"aneg{j}")
                    nc.sync.dma_start(at[:], a_neg[l, j * 128:(j + 1) * 128, :])
                    Asb.append(at)
                if apply_norm_w:
                    nw_sb = wp.tile([128, DM], F32, tag="nwsb")
                    nc.sync.dma_start(nw_sb[:], nwb[l, :, :])
                if apply_norm_b:
                    nb_sb = wp.tile([128, DM], F32, tag="nbsb")
                    nc.sync.dma_start(nb_sb[:], nbb[l, :, :])

                # ---- DRAM staging for this layer ----
                z_st = dram.tile([DL, T], BF16, tag="zst", name=f"zst{l}")
                xdbl_in = [[dram.tile([96, L // 2], F32, tag=f"xdbli{b}_{h}",
                                      name=f"xdbli{l}_{b}_{h}") for h in range(2)]
                           for b in range(B)]
                xdbl_sh = [[dram.tile([96, L // 2], F32, tag=f"xdblo{b}_{h}",
                                      addr_space="Shared",
                                      name=f"xdblo{l}_{b}_{h}") for h in range(2)]
                           for b in range(B)]
                bc_bf = [dram.tile([2 * DS, L], BF16, tag=f"bcbf{b}",
                                   name=f"bcbf{l}_{b}") for b in range(B)]
                a2a_in = [[dram.tile([DI // 2, SLAB], BF16, tag=f"a2ai{b}_{j}",
                                     name=f"a2ai{l}_{b}_{j}") for j in range(NDT)]
                          for b in range(B)]
                a2a_out = [[dram.tile([DI // 2, SLAB], BF16, tag=f"a2ao{b}_{j}",
                                      name=f"a2ao{l}_{b}_{j}") for j in range(NDT)]
                           for b in range(B)]

                # persistent per-layer SBUF
                u_sb = [sp.tile([128, T], BF16, tag=f"usb{j}", name=f"usb{l}_{j}")
                        for j in range(NDT)]
                dt_sb = [sp.tile([128, L], BF16, tag=f"dtsb{j}", name=f"dtsb{l}_{j}")
                         for j in range(NDT)]
                du_sb = [sp.tile([128, L], BF16, tag=f"dusb{j}", name=f"dusb{l}_{j}")
                         for j in range(NDT)]
                prev_uext = [None, None]

                # =========== phase A: LN + transpose + in_proj + conv ===========
                def emit_A(ci):
                    b = ci // 4
                    tok0 = ci * 512
                    hn_t = []
                    if l == 0:
                        nv4 = lnp.tile([128, 4], F32, tag="nv4", bufs=2)
                        mean4 = lnp.tile([128, 4], F32, tag="mean4", bufs=2)
                        for tti in range(4):
                            row0 = b * L + (ci % 4) * 512 + tti * 128
                            xa = lnp.tile([128, DM], F32, tag="xa", bufs=2)
                            nc.sync.dma_start(xa[:], x_dram.ap()[row0:row0 + 128, :])
                            xabf = lnp.tile([128, DM], BF16, tag="xabf", bufs=4)
                            sums = lnp.tile([128, 1], F32, tag="sums", bufs=2)
                            nc.scalar.activation(xabf[:], xa[:], AF.Identity,
                                                 accum_out=sums[:])
                            sqs = lnp.tile([128, DM], F32, tag="sqs", bufs=1)
                            sumsq = lnp.tile([128, 1], F32, tag="sumsq", bufs=2)
                            nc.scalar.activation(sqs[:], xa[:], AF.Square,
                                                 accum_out=sumsq[:])
                            nc.vector.tensor_scalar_mul(
                                mean4[:, tti:tti + 1], sums[:], 1.0 / DM)
                            msq = lnp.tile([128, 1], F32, tag="msq", bufs=2)
                            nc.vector.tensor_scalar_mul(msq[:], sumsq[:], 1.0 / DM)
                            # nv = mean^2 - msq  (negative variance)
                            nc.vector.scalar_tensor_tensor(
                                nv4[:, tti:tti + 1], mean4[:, tti:tti + 1],
                                mean4[:, tti:tti + 1], msq[:], ALU.mult, ALU.subtract)
                            hn_t.append(xabf)
                        # rstd = exp(-0.5*ln(var+eps)) ; var = -nv
                        lnv = lnp.tile([128, 4], F32, tag="lnv", bufs=2)
                        nc.scalar.activation(lnv[:], nv4[:], AF.Ln,
                                             bias=eps_sb[:], scale=-1.0)
                        rstd4 = lnp.tile([128, 4], F32, tag="rstd4", bufs=2)
                        nc.scalar.activation(rstd4[:], lnv[:], AF.Exp, scale=-0.5)
                        nbias4 = lnp.tile([128, 4], F32, tag="nbias4", bufs=2)
                        nc.vector.scalar_tensor_tensor(
                            nbias4[:], mean4[:], -1.0, rstd4[:], ALU.mult, ALU.mult)
                        for tti in range(4):
                            xabf = hn_t[tti]
                            if apply_norm_w or apply_norm_b:
                                hn0 = lnp.tile([128, DM], F32, tag="hn0", bufs=2)
                                nc.scalar.activation(
                                    hn0[:], xabf[:], AF.Identity,
                                    bias=nbias4[:, tti:tti + 1],
                                    scale=rstd4[:, tti:tti + 1])
                                hnn = lnp.tile([128, DM], BF16, tag="hnn", bufs=4)
                                if apply_norm_w and apply_norm_b:
                                    hn1 = lnp.tile([128, DM], F32, tag="hn1", bufs=2)
                                    nc.vector.tensor_mul(hn1[:], hn0[:], nw_sb[:])
                                    nc.vector.tensor_add(hnn[:], hn1[:], nb_sb[:])
                                elif apply_norm_w:
                                    nc.vector.tensor_mul(hnn[:], hn0[:], nw_sb[:])
                                else:
                                    nc.vector.tensor_add(hnn[:], hn0[:], nb_sb[:])
                                hn_t[tti] = hnn
                            else:
                                hnn = lnp.tile([128, DM], BF16, tag="hnn", bufs=4)
                                nc.scalar.activation(
                                    hnn[:], xabf[:], AF.Identity,
                                    bias=nbias4[:, tti:tti + 1],
                                    scale=rstd4[:, tti:tti + 1])
                                hn_t[tti] = hnn
                    else:
                        # layer 2: load pre-normalized tokens from the AllGather
                        for tti in range(4):
                            t0 = (ci % 4) * 512 + tti * 128  # token within batch
                            c = t0 // SLAB
                            row0 = c * SLAB + (t0 % SLAB)
                            hnn = lnp.tile([128, DM], BF16, tag="hnn", bufs=4)
                            nc.sync.dma_start(hnn[:], hn_ag[b][row0:row0 + 128, :])
                            hn_t.append(hnn)

                    # transposes: hnT[kt] = [128 dm, 512 tok] bf16
                    hnT = []
                    for kt in range(8):
                        pt = psA.tile([128, 512], BF16, tag="ptb", bufs=1)
                        for tti in range(4):
                            nc.tensor.transpose(
                                pt[:, tti * 128:(tti + 1) * 128],
                                hn_t[tti][:, kt * 128:(kt + 1) * 128],
                                idbf[:],
                            )
                        ht = lnp.tile([128, 512], BF16, tag=f"hnT{kt}")
                        nc.scalar.copy(ht[:], pt[:])
                        hnT.append(ht)

                    for mt in range(4):
                        pm = psA.tile([128, 512], F32, tag="pm", bufs=2)
                        for kt in range(8):
                            nc.tensor.matmul(
                                pm[:],
                                winT[kt][:, mt * 128:(mt + 1) * 128],
                                hnT[kt][:],
                                start=(kt == 0),
                                stop=(kt == 7),
                            )
                        if mt < NDT:
                            j = mt
                            ue = sp.tile([128, 515], BF16, tag=f"uext{j}", bufs=2)
                            if ci % 4 == 0:
                                nc.vector.memset(ue[:, 0:3], 0.0)
                            else:
                                nc.vector.tensor_copy(
                                    ue[:, 0:3], prev_uext[j][:, 512:515])
                            nc.scalar.copy(ue[:, 3:515], pm[:])
                            prev_uext[j] = ue
                            # conv as 4 diag matmuls accumulated in PSUM
                            cvp = psA.tile([128, 512], F32, tag="cvp", bufs=1)
                            for k in range(DCONV):
                                nc.tensor.matmul(
                                    cvp[:], cdiag[j][k][:], ue[:, k:k + 512],
                                    start=(k == 0), stop=(k == DCONV - 1),
                                )
                            # silu(v) = v * exp(-softplus(-v)), v = cvp + convb
                            vv = sp.tile([128, 512], BF16, tag="vv", bufs=2)
                            nc.scalar.activation(vv[:], cvp[:], AF.Identity,
                                                 bias=convb[j][:])
                            e1 = sp.tile([128, 512], BF16, tag="e1", bufs=1)
                            nc.scalar.activation(e1[:], vv[:], AF.Exp, scale=-1.0)
                            sp1 = sp.tile([128, 512], BF16, tag="sp1", bufs=1)
                            nc.scalar.activation(sp1[:], e1[:], AF.Ln,
                                                 bias=one_sb[:, 0:1])
                            sg = sp.tile([128, 512], BF16, tag="sg", bufs=2)
                            nc.scalar.activation(sg[:], sp1[:], AF.Exp, scale=-1.0)
                            nc.vector.tensor_mul(
                                u_sb[j][:, tok0:tok0 + 512], vv[:], sg[:])
                            if j == 0:
                                px_hold[0] = psA.tile([96, 512], F32, tag="px",
                                                      bufs=1, name=f"px{l}_{ci}")
                            px = px_hold[0]
                            nc.tensor.matmul(
                                px[:], wxpT[j][:], u_sb[j][:, tok0:tok0 + 512],
                                start=(j == 0), stop=(j == NDT - 1),
                            )
                        else:
                            j = mt - NDT
                            vv = sp.tile([128, 512], BF16, tag="vv", bufs=2)
                            nc.scalar.copy(vv[:], pm[:])
                            e1 = sp.tile([128, 512], BF16, tag="e1", bufs=1)
                            nc.scalar.activation(e1[:], vv[:], AF.Exp, scale=-1.0)
                            sp1 = sp.tile([128, 512], BF16, tag="sp1", bufs=1)
                            nc.scalar.activation(sp1[:], e1[:], AF.Ln,
                                                 bias=one_sb[:, 0:1])
                            sg = sp.tile([128, 512], BF16, tag="sg", bufs=2)
                            nc.scalar.activation(sg[:], sp1[:], AF.Exp, scale=-1.0)
                            zc = sp.tile([128, 512], BF16, tag="zc", bufs=2)
                            nc.vector.tensor_mul(zc[:], vv[:], sg[:])
                            nc.sync.dma_start(
                                z_st[j * 128:(j + 1) * 128, tok0:tok0 + 512], zc[:])
                    pxs = sp.tile([96, 512], F32, tag="pxs", bufs=2)
                    nc.scalar.copy(pxs[:], px_hold[0][:])
                    nc.sync.dma_start(
                        xdbl_in[ci // 4][(ci % 4) // 2][:, (ci % 2) * 512:(ci % 2) * 512 + 512],
                        pxs[:])

                def emit_AR(b, h):
                    all_reduce(xdbl_in[b][h].opt(), xdbl_sh[b][h].opt())

                def emit_loadback(b, h):
                    xbf = dp.tile([96, L // 2], F32, tag=f"xbf{h}", bufs=1,
                                  name=f"xbf{l}_{b}_{h}")
                    nc.sync.dma_start(xbf[:], xdbl_sh[b][h][:, :])
                    xrd = dp.tile([DTR, L // 2], BF16, tag=f"xrd{h}", bufs=1,
                                  name=f"xrd{l}_{b}_{h}")
                    nc.scalar.copy(xrd[:], xbf[0:DTR, :])
                    bcb = dp.tile([2 * DS, L // 2], BF16, tag=f"bcb{h}", bufs=1,
                                  name=f"bcb{l}_{b}_{h}")
                    nc.scalar.copy(bcb[:], xbf[DTR:96, :])
                    nc.sync.dma_start(
                        bc_bf[b][:, h * (L // 2):(h + 1) * (L // 2)], bcb[:])
                    return xrd

                def emit_D(b, j, h, xrd):
                    # dt = softplus(dt_proj(dt_r) + dt_b);  du = dt * u
                    h0 = h * (L // 2)
                    for q in range(2):
                        pd = psA.tile([128, 512], F32, tag="cvp", bufs=1)
                        nc.tensor.matmul(
                            pd[:],
                            wdtT[:, j * 128:(j + 1) * 128],
                            xrd[:, q * 512:(q + 1) * 512],
                            start=True, stop=True,
                        )
                        ef = dp.tile([128, 512], BF16, tag="ef", bufs=1)
                        nc.scalar.activation(ef[:], pd[:], AF.Exp, bias=dtb[j][:])
                        nc.scalar.activation(
                            dt_sb[j][:, h0 + q * 512:h0 + (q + 1) * 512], ef[:],
                            AF.Ln, bias=one_sb[:, 0:1])
                    nc.vector.tensor_mul(
                        du_sb[j][:, h0:h0 + L // 2],
                        dt_sb[j][:, h0:h0 + L // 2],
                        u_sb[j][:, b * L + h0:b * L + h0 + L // 2])

                y_ps_hold = [None]
                px_hold = [None]
                hl_hold = [None]

                def emit_E(b, j, h, n0, n1):
                    LH = L // 2
                    h0 = h * LH
                    if n0 == 0:
                        y_ps_hold[0] = psE.tile([128, LH], F32, tag="yps", bufs=1,
                                                name=f"yps{l}_{b}_{j}_{h}")
                        if h == 0:
                            hl_hold[0] = dp.tile([128, DS], F32, tag="hl", bufs=2,
                                                 name=f"hl{l}_{b}_{j}")
                    y_ps = y_ps_hold[0]
                    hl = hl_hold[0]
                    for n in range(n0, n1):
                        pb = dp.tile([128, LH], BF16, tag="pb", bufs=3)
                        nc.sync.dma_start(
                            pb[:],
                            bc_bf[b][n:n + 1, h0:h0 + LH].to_broadcast((128, LH)))
                        pc = dp.tile([128, LH], BF16, tag="pc", bufs=3)
                        nc.sync.dma_start(
                            pc[:],
                            bc_bf[b][DS + n:DS + n + 1, h0:h0 + LH]
                            .to_broadcast((128, LH)))
                        ada = dp.tile([128, LH], BF16, tag="ada", bufs=2)
                        nc.scalar.activation(
                            ada[:], dt_sb[j][:, h0:h0 + LH], AF.Exp,
                            scale=Asb[j][:, n:n + 1])
                        bt = dp.tile([128, LH], BF16, tag="bt", bufs=3)
                        nc.vector.tensor_mul(
                            bt[:], du_sb[j][:, h0:h0 + LH], pb[:])
                        hs = dp.tile([128, LH], BF16, tag="hs", bufs=1)
                        nc.vector.tensor_tensor_scan(
                            hs[:], ada[:], bt[:],
                            0.0 if h == 0 else hl[:, n:n + 1],
                            ALU.mult, ALU.add)
                        if h == 0:
                            nc.scalar.copy(hl[:, n:n + 1], hs[:, LH - 1:LH])
                        yt = dp.tile([128, LH], BF16, tag="yt", bufs=2)
                        nc.vector.tensor_mul(yt[:], hs[:], pc[:])
                        for q in range(2):
                            nc.tensor.matmul(
                                y_ps[:, q * 512:(q + 1) * 512],
                                idbf[:],
                                yt[:, q * 512:(q + 1) * 512],
                                start=(n == 0),
                                stop=(n == DS - 1),
                            )

                def emit_Fgate(b, j, h):
                    LH = L // 2
                    h0 = h * LH
                    y_ps = y_ps_hold[0]
                    ysb = dp.tile([128, LH], BF16, tag="ysb", bufs=2)
                    nc.scalar.copy(ysb[:], y_ps[:])
                    y1 = dp.tile([128, LH], BF16, tag="y1g", bufs=2)
                    nc.vector.scalar_tensor_tensor(
                        y1[:], u_sb[j][:, b * L + h0:b * L + h0 + LH], Dp[j][:],
                        ysb[:], ALU.mult, ALU.add)
                    zb = dp.tile([128, LH], BF16, tag="zb", bufs=2)
                    nc.sync.dma_start(
                        zb[:], z_st[j * 128:(j + 1) * 128,
                                    b * L + h0:b * L + h0 + LH])
                    yg = dp.tile([128, LH], BF16, tag="yg", bufs=2)
                    nc.vector.tensor_mul(yg[:], y1[:], zb[:])
                    # stage for per-j AllToAll: block c' = rows [c'*128, +128)
                    for c in range(4):
                        cs = h * 4 + c
                        nc.sync.dma_start(
                            a2a_in[b][j][cs * 128:(cs + 1) * 128, :],
                            yg[:, c * SLAB:(c + 1) * SLAB],
                        )
                    if h == 1:
                        all_to_all(a2a_in[b][j].opt(), a2a_out[b][j].opt())

                def emit_F(b):
                    # global channel tile k lives in a2a_out[b][k % 2], block k // 2
                    yall = []
                    for k in range(16):
                        yk = dp.tile([128, SLAB], BF16, tag=f"ya{k}", bufs=1,
                                     name=f"ya{l}_{b}_{k}")
                        nc.sync.dma_start(
                            yk[:],
                            a2a_out[b][k % 2][(k // 2) * 128:(k // 2 + 1) * 128, :])
                        yall.append(yk)
                    for tt in range(2):
                        po = [psA.tile([128, 512], F32, tag="cvp", bufs=1,
                                       name=f"po0_{l}_{b}_{tt}"),
                              psA.tile([128, 512], F32, tag="pm", bufs=2,
                                       name=f"po1_{l}_{b}_{tt}")]
                        for k in range(16):
                            for hh in range(2):
                                nc.tensor.matmul(
                                    po[hh][:],
                                    yall[k][:, tt * 128:(tt + 1) * 128],
                                    wout[k][:, hh * 512:(hh + 1) * 512],
                                    start=(k == 0),
                                    stop=(k == 15),
                                )
                        slab = dp.tile([128, DM], F32, tag="slab", bufs=1)
                        nc.any.tensor_copy(slab[:, 0:512], po[0][:])
                        nc.any.tensor_copy(slab[:, 512:DM], po[1][:])
                        if l == DEPTH - 1:
                            nc.sync.dma_start(
                                out_dram[b * SLAB + tt * 128: b * SLAB + (tt + 1) * 128, :],
                                slab[:],
                            )
                        else:
                            # LayerNorm the slab for the next layer, then stage
                            sums = dp.tile([128, 1], F32, tag="ssum", bufs=2)
                            sscr = dp.tile([128, DM], BF16, tag="sscr", bufs=1)
                            nc.scalar.activation(sscr[:], slab[:], AF.Identity,
                                                 accum_out=sums[:])
                            sumsq = dp.tile([128, 1], F32, tag="ssq", bufs=2)
                            nc.scalar.activation(sscr[:], slab[:], AF.Square,
                                                 accum_out=sumsq[:])
                            mean = dp.tile([128, 1], F32, tag="smean", bufs=2)
                            nc.vector.tensor_scalar_mul(mean[:], sums[:], 1.0 / DM)
                            msq = dp.tile([128, 1], F32, tag="smsq", bufs=2)
                            nc.vector.tensor_scalar_mul(msq[:], sumsq[:], 1.0 / DM)
                            nv = dp.tile([128, 1], F32, tag="snv", bufs=2)
                            nc.vector.scalar_tensor_tensor(
                                nv[:], mean[:], mean[:], msq[:], ALU.mult, ALU.subtract)
                            lnv = dp.tile([128, 1], F32, tag="slnv", bufs=2)
                            nc.scalar.activation(lnv[:], nv[:], AF.Ln,
                                                 bias=eps_sb[:], scale=-1.0)
                            rstd = dp.tile([128, 1], F32, tag="srstd", bufs=2)
                            nc.scalar.activation(rstd[:], lnv[:], AF.Exp, scale=-0.5)
                            nbias = dp.tile([128, 1], F32, tag="snb", bufs=2)
                            nc.vector.scalar_tensor_tensor(
                                nbias[:], mean[:], -1.0, rstd[:], ALU.mult, ALU.mult)
                            if apply_norm_w or apply_norm_b:
                                hn0 = dp.tile([128, DM], F32, tag="shn0", bufs=2)
                                nc.scalar.activation(hn0[:], slab[:], AF.Identity,
                                                     bias=nbias[:], scale=rstd[:])
                                hnn = dp.tile([128, DM], BF16, tag="shnn", bufs=2)
                                if apply_norm_w and apply_norm_b:
                                    hn1 = dp.tile([128, DM], F32, tag="shn1", bufs=2)
                                    nc.vector.tensor_mul(hn1[:], hn0[:], nw2_sb[:])
                                    nc.vector.tensor_add(hnn[:], hn1[:], nb2_sb[:])
                                elif apply_norm_w:
                                    nc.vector.tensor_mul(hnn[:], hn0[:], nw2_sb[:])
                                else:
                                    nc.vector.tensor_add(hnn[:], hn0[:], nb2_sb[:])
                            else:
                                hnn = dp.tile([128, DM], BF16, tag="shnn", bufs=1)
                                nc.scalar.activation(hnn[:], slab[:], AF.Identity,
                                                     bias=nbias[:], scale=rstd[:])
                            nc.sync.dma_start(
                                hn_ag_in[b][tt * 128:(tt + 1) * 128, :],
                                hnn[:],
                            )

                # next-layer norm weights for the slab-LN
                if l < DEPTH - 1 and apply_norm_w:
                    nw2_sb = wp.tile([128, DM], F32, tag="nw2sb")
                    nc.sync.dma_start(nw2_sb[:], nwb[l + 1, :, :])
                if l < DEPTH - 1 and apply_norm_b:
                    nb2_sb = wp.tile([128, DM], F32, tag="nb2sb")
                    nc.sync.dma_start(nb2_sb[:], nbb[l + 1, :, :])

                # ================= emission schedule =================
                emit_A(0); emit_A(1)
                emit_AR(0, 0)
                xrd00 = emit_loadback(0, 0)
                emit_A(2)
                emit_D(0, 0, 0, xrd00)
                emit_D(0, 1, 0, xrd00)
                emit_E(0, 0, 0, 0, 8)
                emit_A(3)
                emit_AR(0, 1)
                xrd01 = emit_loadback(0, 1)
                emit_E(0, 0, 0, 8, 16)
                emit_Fgate(0, 0, 0)
                emit_A(4)
                emit_D(0, 0, 1, xrd01)
                emit_E(0, 0, 1, 0, 16)
                emit_Fgate(0, 0, 1)
                emit_A(5)
                emit_AR(1, 0)
                xrd10 = emit_loadback(1, 0)
                emit_E(0, 1, 0, 0, 16)
                emit_Fgate(0, 1, 0)
                emit_A(6)
                emit_D(0, 1, 1, xrd01)
                emit_E(0, 1, 1, 0, 16)
                emit_Fgate(0, 1, 1)
                emit_A(7)
                emit_AR(1, 1)
                xrd11 = emit_loadback(1, 1)
                emit_D(1, 0, 0, xrd10)
                emit_E(1, 0, 0, 0, 16)
                emit_Fgate(1, 0, 0)
                emit_F(0)
                if l < DEPTH - 1:
                    all_gather(hn_ag_in[0].opt(), hn_ag[0].opt())
                emit_D(1, 0, 1, xrd11)
                emit_E(1, 0, 1, 0, 16)
                emit_Fgate(1, 0, 1)
                emit_D(1, 1, 0, xrd10)
                emit_E(1, 1, 0, 0, 16)
                emit_Fgate(1, 1, 0)
                emit_D(1, 1, 1, xrd11)
                emit_E(1, 1, 1, 0, 16)
                emit_Fgate(1, 1, 1)
                emit_F(1)
                if l < DEPTH - 1:
                    all_gather(hn_ag_in[1].opt(), hn_ag[1].opt())

    nc.compile()
    return nc


_CACHE = {}


def _get_nc(apply_norm_w, apply_norm_b, fake_cc=False):
    key = (apply_norm_w, apply_norm_b, fake_cc)
    if key not in _CACHE:
        _CACHE[key] = build_nc(apply_norm_w, apply_norm_b, fake_cc)
    return _CACHE[key]


def _bf16(a):
    import jax.numpy as jnp
    return np.asarray(jnp.asarray(np.asarray(a, np.float32), jnp.bfloat16))


def make_in_maps(x, norm_w, norm_b, in_proj_w, conv_w, conv_b, x_proj_w,
                 dt_proj_w, dt_proj_b, A_log, D, out_proj_w,
                 apply_norm_w, apply_norm_b):
    f = lambda a: np.ascontiguousarray(np.asarray(a), dtype=np.float32)
    x_tm = f(x).reshape(T, DM)
    in_proj_w = np.asarray(in_proj_w)
    conv_w = np.asarray(conv_w)
    a_neg = -np.exp(np.asarray(A_log, np.float64)).astype(np.float32)
    out_T = f(np.asarray(out_proj_w).transpose(0, 2, 1))  # [DEPTH, DI, DM]
    in_maps = []
    for c in range(NCORES):
        sl = slice(c * DL, (c + 1) * DL)
        w_in_rows = np.concatenate(
            [in_proj_w[:, sl, :], in_proj_w[:, DI + c * DL: DI + (c + 1) * DL, :]],
            axis=1,
        )  # (2, 512, 1024)
        cw = f(conv_w[:, sl, 0, :])  # [DEPTH, DL, DCONV]
        cdg = np.zeros((DEPTH, NDT, DCONV, 128, 128), np.float32)
        for li in range(DEPTH):
            for j in range(NDT):
                for k in range(DCONV):
                    np.fill_diagonal(cdg[li, j, k], cw[li, j * 128:(j + 1) * 128, k])
        cb = f(np.asarray(conv_b)[:, sl][..., None])
        m = {
            "x_tm": x_tm,
            "w_inT": _bf16(w_in_rows.transpose(0, 2, 1)),
            "w_outF": _bf16(out_T),
            "w_xpT": _bf16(np.asarray(x_proj_w)[:, :, sl].transpose(0, 2, 1)),
            "w_dtT": _bf16(np.asarray(dt_proj_w)[:, sl, :].transpose(0, 2, 1)),
            "conv_dg": _bf16(cdg),
            "conv_b_c": cb,
            "nconv_b_c": -cb,
            "dt_b_c": f(np.asarray(dt_proj_b)[:, sl][..., None]),
            "a_neg_c": f(a_neg[:, sl, :]),
            "d_c": f(np.asarray(D)[:, sl][..., None]),
            "ident_bf": _bf16(np.eye(128, dtype=np.float32)),
        }
        if apply_norm_w:
            m["norm_w_bc"] = f(np.broadcast_to(
                np.asarray(norm_w)[:, None, :], (DEPTH, 128, DM)))
        if apply_norm_b:
            m["norm_b_bc"] = f(np.broadcast_to(
                np.asarray(norm_b)[:, None, :], (DEPTH, 128, DM)))
        in_maps.append(m)
    return in_maps


def kernel(x, x_size, norm_w, norm_b, in_proj_w, conv_w, conv_b, x_proj_w,
           dt_proj_w, dt_proj_b, A_log, D, out_proj_w, **_unused):
    apply_norm_w = not np.allclose(np.asarray(norm_w), 1.0)
    apply_norm_b = not np.allclose(np.asarray(norm_b), 0.0)
    nc = _get_nc(apply_norm_w, apply_norm_b)
    in_maps = make_in_maps(
        x, norm_w, norm_b, in_proj_w, conv_w, conv_b, x_proj_w,
        dt_proj_w, dt_proj_b, A_log, D, out_proj_w,
        apply_norm_w, apply_norm_b,
    )
    res = run_bass_kernel_spmd(nc, in_maps, core_ids=list(range(NCORES)))
    out = np.zeros((B, L, DM), np.float32)
    for c in range(NCORES):
        s = np.asarray(res.results[c]["out_tm"])
        for b in range(B):
            out[b, c * SLAB:(c + 1) * SLAB, :] = s[b * SLAB:(b + 1) * SLAB, :]
    return out
